# revision 3
# baseline (speedup 1.0000x reference)
"""Trainium2 Bass kernel v2: 2-layer mean-aggregation GraphSAGE encoder.

Wave-pipelined combined gather. Nodes (relabeled by degree-balanced binning
into 8 cores x 49 tiles of 128) are processed in W waves of ~4 tiles. A
DRAM table xq holds one 1KB bf16 row per node PAIR: [x(2i)|x(2i+1)|q(2i)|
q(2i+1)]; the q halves are filled during execution as each wave's layer-1
output q = h @ W2n.T is AllGather'd and repacked in.

Per dst tile, edges are split into EARLY (src wave <= dst wave - lag, q
already available) and LATE. Pass 1 gathers, per edge slot, the full 1KB
pair-row for early edges (x half feeds the layer-1 one-hot segment-sum, q
half feeds layer-2 partial aggregation with the SAME one-hot) but only the
512B x-half for late edges. Pass 2 gathers only the late edges' 512B
q-half. SWDGE descriptor generation on the Pool engine (~8ns/descriptor,
the kernel's critical resource) thus drops from 2E to (1 + late)E per core,
and gathered bytes stay under the DMA engines' pacing threshold.

A slot's one-hot column encodes dst-in-tile + 128*parity(src), so one
[128,256] DVE is_equal per chunk yields both the even-src and odd-src
one-hots. Layer-2 partial sums are stashed unscaled in SBUF (bf16) and
added back in pass 2 via an identity matmul into PSUM before the deg_inv
scale. AllGather latency (~55us end-to-end, strictly serialized on the CC
cores) is hidden by sizing waves >= that latency and triggering each AG
mid-next-wave; repacks run two waves after their AG.

All index bookkeeping is host-side on edge_index only; feature FLOPs run on
the NeuronCores in bf16 with fp32 PSUM accumulation.
"""

import numpy as np
import ml_dtypes

import concourse.bass as bass
from concourse import bacc, mybir, tile
from concourse import bass_utils

P = 128
F32 = mybir.dt.float32
BF16 = mybir.dt.bfloat16
I16 = mybir.dt.int16
BF = ml_dtypes.bfloat16


def rup(v):
    return (-(-np.asarray(v) // P) * P).astype(np.int64)


class Cfg:
    def __init__(self, n=50000, n_cores=8, in_dim=128, hid=256, out_dim=128,
                 wave_sizes=None, lag=3):
        assert n % n_cores == 0
        self.n = n
        self.n_cores = n_cores
        self.in_dim, self.hid, self.out_dim = in_dim, hid, out_dim
        self.npc = n // n_cores
        self.nt = -(-self.npc // P)          # 49 dst tiles per core
        self.npad = self.nt * P              # 6272
        self.npos = n_cores * self.npad      # 50176
        self.npair = self.npos // 2
        if wave_sizes is None:
            wave_sizes = [4] * (self.nt // 4)
            if self.nt % 4:
                wave_sizes[-1] += self.nt % 4
        assert sum(wave_sizes) == self.nt
        self.W = len(wave_sizes)
        self.lag = lag
        self.wave_of = np.repeat(np.arange(self.W), wave_sizes)
        self.wave_tiles = [np.nonzero(self.wave_of == w)[0]
                           for w in range(self.W)]
        # gather groups: consecutive tiles, <= 2 per group, within one wave
        self.groups = []
        for w in range(self.W):
            wt = [int(t) for t in self.wave_tiles[w]]
            for i in range(0, len(wt), 2):
                self.groups.append(wt[i:i + 2])
        self.newpos = None

    def key(self):
        return (self.n, self.n_cores, self.W, self.lag)


def _rank_in_groups(seg, nseg):
    start = np.searchsorted(seg, np.arange(nseg))
    return np.arange(len(seg)) - start[seg]


def pass2_order_groups(nt, B2):
    """Tiles in descending-B2 order (smallest last, to shrink the kernel
    tail), grouped into <=6144-slot gathers; returns (order, groups,
    base2) with base2 giving each tile's slot offset in that layout."""
    order = sorted(range(nt), key=lambda t: (-int(B2[t]), t))
    groups = []
    gi = 0
    while gi < nt:
        grp = [order[gi]]
        while (gi + len(grp) < nt and len(grp) < 4
               and sum(int(B2[u]) for u in grp)
               + int(B2[order[gi + len(grp)]]) <= 6144):
            grp.append(order[gi + len(grp)])
        groups.append(grp)
        gi += len(grp)
    if len(groups[-1]) > 1:
        last = groups[-1]
        groups[-1] = last[:-1]
        groups.append(last[-1:])
    base2 = np.zeros(nt + 1, np.int64)
    off = 0
    for t in order:
        base2[t] = off
        off += int(B2[t])
    base2[nt] = off
    return order, groups, base2


def slot_layout(cfg, Be, Bl):
    """Global slot ranges for pass 1: per gather-group, early slots of its
    tiles (each tile's range rounded to 128), then late slots likewise.
    Returns (ebase[t], lbase[t], group spans, tot1)."""
    BeR, BlR = rup(Be), rup(Bl)
    ebase = np.zeros(cfg.nt, np.int64)
    lbase = np.zeros(cfg.nt, np.int64)
    spans = []          # per group: (e0, esz, l0, lsz)
    off = 0
    for grp in cfg.groups:
        e0 = off
        for t in grp:
            ebase[t] = off
            off += int(BeR[t])
        l0 = off
        for t in grp:
            lbase[t] = off
            off += int(BlR[t])
        spans.append((e0, l0 - e0, l0, off - l0))
    return ebase, lbase, spans, off


def pack_inputs(cfg, x, edge_index, w1n, w1r, b1, w2n, w2r, b2):
    x = np.asarray(x, np.float32)
    src = np.asarray(edge_index[0], np.int64)
    dst = np.asarray(edge_index[1], np.int64)
    n, nt, npad = cfg.n, cfg.nt, cfg.npad

    deg = np.bincount(dst, minlength=n).astype(np.float32)
    deginv = np.where(deg > 0,
                      np.float32(1.0) / np.maximum(deg, np.float32(1.0)),
                      np.float32(0.0)).astype(np.float32)

    # degree-balanced serpentine assignment of dst nodes to (core,tile) bins
    nbins = cfg.n_cores * nt
    order = np.argsort(-deg, kind="stable")
    r = np.arange(n)
    p_pass = r // nbins
    j = r % nbins
    b = np.where(p_pass % 2 == 0, j, nbins - 1 - j)
    newpos = np.empty(n, np.int64)
    newpos[order] = b * P + p_pass
    cfg.newpos = newpos

    spos = newpos[src]
    dpos = newpos[dst]
    core = dpos // npad
    tl = (dpos % npad) // P
    di = (dpos % P).astype(np.int64)
    par = spos & 1
    pairidx = spos >> 1
    wsrc = cfg.wave_of[(spos % npad) // P]
    wdst = cfg.wave_of[tl]
    early = wsrc <= wdst - cfg.lag
    dval = (di + P * par).astype(np.float32)

    key = core * nt + tl
    cnt_e = np.bincount(key[early], minlength=cfg.n_cores * nt).reshape(
        cfg.n_cores, nt)
    cnt_l = np.bincount(key[~early], minlength=cfg.n_cores * nt).reshape(
        cfg.n_cores, nt)
    Be = cnt_e.max(0)
    Bl = np.maximum(cnt_l.max(0), 1)
    ebase, lbase, spans, tot1 = slot_layout(cfg, Be, Bl)
    B2 = rup(Bl)
    _, g2grps, base2 = pass2_order_groups(nt, B2)
    tot2 = int(base2[nt])

    slotnode = np.full(cfg.npos, -1, np.int64)
    slotnode[newpos] = np.arange(n)
    m_sl = slotnode >= 0

    # xq pair rows: [x(2i) | x(2i+1) | q(2i) | q(2i+1)]
    xq4 = np.zeros((cfg.npair, 4, P), BF)
    xn = np.zeros((cfg.npos, P), BF)
    xn[m_sl] = x[slotnode[m_sl]].astype(BF)
    xq4[:, 0, :] = xn[0::2]
    xq4[:, 1, :] = xn[1::2]
    xq = np.ascontiguousarray(xq4.reshape(cfg.npair, 4 * P))

    w1n = np.asarray(w1n, np.float32)
    w1r = np.asarray(w1r, np.float32)
    w2n = np.asarray(w2n, np.float32)
    w2r = np.asarray(w2r, np.float32)
    iota = np.tile(np.arange(2 * P, dtype=np.float32), (P, 1))
    shared = {
        "xq": xq,
        "w1nT": np.ascontiguousarray(w1n.T.astype(BF)),
        "w1rT": np.ascontiguousarray(w1r.T.astype(BF)),
        "b1c": np.ascontiguousarray(np.asarray(b1, np.float32).reshape(2, P).T),
        "w2nT": np.ascontiguousarray(
            np.concatenate([w2n.T[:P, :], w2n.T[P:, :]], axis=1).astype(BF)),
        "w2rT": np.ascontiguousarray(
            np.concatenate([w2r.T[:P, :], w2r.T[P:, :]], axis=1).astype(BF)),
        "b2r": np.ascontiguousarray(np.asarray(b2, np.float32).reshape(1, P)
                                    .astype(BF)),
        "iota2": np.ascontiguousarray(iota.astype(BF)),
        "ident": np.ascontiguousarray(np.eye(P, dtype=np.float32).astype(BF)),
    }

    in_maps = []
    for c in range(cfg.n_cores):
        m = core == c
        etl, edi = tl[m], dval[m]
        epair, eearly = pairidx[m], early[m]

        so = np.argsort(etl * 2 + (~eearly).astype(np.int64), kind="stable")
        s_tl, s_early = etl[so], eearly[so]
        seg = s_tl * 2 + (~s_early).astype(np.int64)
        rank = _rank_in_groups(seg, 2 * nt)
        slot1 = np.where(s_early, ebase[s_tl] + rank, lbase[s_tl] + rank)
        eidx1 = np.zeros(tot1, np.int64)
        dloc1 = np.full(tot1, -1.0, np.float32)
        BeR_, BlR_ = rup(Be), rup(Bl)
        for grp in cfg.groups:
            tl_ = grp[-1]
            eidx1[ebase[tl_] + Be[tl_]:ebase[tl_] + BeR_[tl_]] = -1
            eidx1[lbase[tl_] + Bl[tl_]:lbase[tl_] + BlR_[tl_]] = -1
        eidx1[slot1] = epair[so]
        dloc1[slot1] = edi[so]

        lm = ~s_early
        slot2 = base2[s_tl[lm]] + rank[lm]
        eidx2 = np.zeros(tot2, np.int64)
        dloc2 = np.full(tot2, -1.0, np.float32)
        for grp in g2grps:
            tl_ = grp[-1]
            eidx2[base2[tl_] + Bl[tl_]:base2[tl_] + B2[tl_]] = -1
        eidx2[slot2] = epair[so][lm]
        dloc2[slot2] = edi[so][lm]

        sl = slotnode[c * npad:(c + 1) * npad]
        msl = sl >= 0
        xt = np.zeros((P, npad), BF)
        xt[:, msl] = x[sl[msl]].T.astype(BF)
        dinv = np.zeros((P, nt), np.float32)
        dinv.T.flat[msl] = deginv[sl[msl]]

        im = dict(shared)
        im["eidx1"] = np.ascontiguousarray(
            np.tile(eidx1.astype(np.int16).reshape(-1, 16).T, (8, 1)))
        im["dloc1"] = np.ascontiguousarray(dloc1.reshape(-1, P).T.astype(BF))
        im["eidx2"] = np.ascontiguousarray(
            np.tile(eidx2.astype(np.int16).reshape(-1, 16).T, (8, 1)))
        im["dloc2"] = np.ascontiguousarray(dloc2.reshape(-1, P).T.astype(BF))
        im["xtile"] = xt
        im["dinv"] = dinv
        in_maps.append(im)

    budgets = (tuple(int(v) for v in Be), tuple(int(v) for v in Bl))
    return budgets, in_maps


# --------------------------------------------------------------------------
# Bass program
# --------------------------------------------------------------------------

def build_program(cfg, budgets):
    Be = np.array(budgets[0], np.int64)
    Bl = np.array(budgets[1], np.int64)
    BeR, BlR = rup(Be), rup(Bl)
    ebase, lbase, spans, tot1 = slot_layout(cfg, Be, Bl)
    B2 = rup(Bl)
    nt, npad, W = cfg.nt, cfg.npad, cfg.W
    _, _g2, base2 = pass2_order_groups(nt, B2)
    tot2 = int(base2[nt])

    nc = bacc.Bacc("TRN2", target_bir_lowering=False, debug=False,
                   enable_asserts=False, num_devices=cfg.n_cores)

    xq_d = nc.dram_tensor("xq", [cfg.npair, 4 * P], BF16, kind="ExternalInput")
    e1_d = nc.dram_tensor("eidx1", [P, tot1 // 16], I16, kind="ExternalInput")
    d1_d = nc.dram_tensor("dloc1", [P, tot1 // P], BF16, kind="ExternalInput")
    e2_d = nc.dram_tensor("eidx2", [P, tot2 // 16], I16, kind="ExternalInput")
    d2_d = nc.dram_tensor("dloc2", [P, tot2 // P], BF16, kind="ExternalInput")
    xt_d = nc.dram_tensor("xtile", [P, npad], BF16, kind="ExternalInput")
    dinv_d = nc.dram_tensor("dinv", [P, nt], F32, kind="ExternalInput")
    w1n_d = nc.dram_tensor("w1nT", [P, cfg.hid], BF16, kind="ExternalInput")
    w1r_d = nc.dram_tensor("w1rT", [P, cfg.hid], BF16, kind="ExternalInput")
    b1_d = nc.dram_tensor("b1c", [P, 2], F32, kind="ExternalInput")
    w2n_d = nc.dram_tensor("w2nT", [P, 2 * P], BF16, kind="ExternalInput")
    w2r_d = nc.dram_tensor("w2rT", [P, 2 * P], BF16, kind="ExternalInput")
    b2_d = nc.dram_tensor("b2r", [1, P], BF16, kind="ExternalInput")
    iota_d = nc.dram_tensor("iota2", [P, 2 * P], BF16, kind="ExternalInput")
    id_d = nc.dram_tensor("ident", [P, P], BF16, kind="ExternalInput")
    out_d = nc.dram_tensor("out", [npad, P], F32, kind="ExternalOutput")

    AF = mybir.ActivationFunctionType
    OP = mybir.AluOpType

    # max gather-buffer size (bf16 elems per partition): early rows (4P
    # elems each) plus late x-halves (2P) share one pooled buffer
    max_g1 = max(es * 4 + ls * 2 for (_, es, _, ls) in spans)
    _, g2sizes, base2 = pass2_order_groups(nt, B2)
    max_g2 = max(sum(int(B2[u]) for u in grp) for grp in g2sizes) * 2

    with tile.TileContext(nc) as tc:
        with (tc.tile_pool(name="const", bufs=1) as cp,
              tc.tile_pool(name="dram", bufs=1, space="DRAM") as dp,
              tc.tile_pool(name="oh", bufs=6) as ohp,
              tc.tile_pool(name="stage", bufs=4) as sp):
            eidx1 = cp.tile([P, tot1 // 16], I16)
            nc.sync.dma_start(out=eidx1[:], in_=e1_d.ap())
            dloc1 = cp.tile([P, tot1 // P], BF16)
            nc.sync.dma_start(out=dloc1[:], in_=d1_d.ap())
            iota2 = cp.tile([P, 2 * P], BF16)
            nc.sync.dma_start(out=iota2[:], in_=iota_d.ap())
            ident = cp.tile([P, P], BF16)
            nc.sync.dma_start(out=ident[:], in_=id_d.ap())
            xt = cp.tile([P, npad], BF16)
            nc.sync.dma_start(out=xt[:], in_=xt_d.ap())
            dinv = cp.tile([P, nt], F32)
            nc.sync.dma_start(out=dinv[:], in_=dinv_d.ap())
            w1n = cp.tile([P, cfg.hid], BF16)
            nc.sync.dma_start(out=w1n[:], in_=w1n_d.ap())
            w1r = cp.tile([P, cfg.hid], BF16)
            nc.sync.dma_start(out=w1r[:], in_=w1r_d.ap())
            b1c = cp.tile([P, 2], F32)
            nc.sync.dma_start(out=b1c[:], in_=b1_d.ap())
            w2n = cp.tile([P, 2 * P], BF16)
            nc.sync.dma_start(out=w2n[:], in_=w2n_d.ap())
            w2r = cp.tile([P, 2 * P], BF16)
            nc.sync.dma_start(out=w2r[:], in_=w2r_d.ap())
            b2r = cp.tile([1, P], BF16)
            nc.sync.dma_start(out=b2r[:], in_=b2_d.ap())
            eidx2 = cp.tile([P, tot2 // 16], I16)
            nc.sync.dma_start(out=eidx2[:], in_=e2_d.ap())
            dloc2 = cp.tile([P, tot2 // P], BF16)
            nc.sync.dma_start(out=dloc2[:], in_=d2_d.ap())
            ones1 = cp.tile([1, P], BF16)
            nc.vector.memset(ones1[:], 1.0)
            hT = cp.tile([P, 2 * npad], BF16)
            stash = cp.tile([P, nt * P], BF16)

            warm_in = dp.tile([P, P], BF16, name="warm_in")
            warm_out = dp.tile([cfg.n_cores * P, P], BF16, name="warm_out",
                               addr_space="Shared")
            q_locs, q_alls = [], []
            for w in range(W):
                rows = len(cfg.wave_tiles[w]) * P
                q_locs.append(dp.tile([rows, P], BF16, name=f"qloc{w}"))
                q_alls.append(dp.tile([cfg.n_cores * rows, P], BF16,
                                      name=f"qall{w}", addr_space="Shared"))

            def onehot(dloc_t, gc):
                oh = ohp.tile([P, 2 * P], BF16, tag="oh")
                nc.vector.tensor_tensor(
                    out=oh[:], in0=iota2[:],
                    in1=dloc_t[:, gc:gc + 1].to_broadcast([P, 2 * P]),
                    op=OP.is_equal)
                return oh

            def emit_repack(w):
                """q_all[w] rows -> q-halves of xq pair rows (both parities).
                q(pos) lives at pair pos>>1, elem offset (2+(pos&1))*128."""
                rows = len(cfg.wave_tiles[w]) * P
                t0 = int(cfg.wave_tiles[w][0])
                pr0 = t0 * P // 2
                npr = rows // 2
                qa = q_alls[w][:, :].rearrange("(c j t) e -> c j (t e)",
                                               c=cfg.n_cores, t=2)
                xqv = xq_d.ap().rearrange("(c q) e -> c q e", c=cfg.n_cores)
                for parity in range(2):
                    nc.sync.dma_start(
                        out=xqv[:, pr0:pr0 + npr,
                                (2 + parity) * P:(3 + parity) * P],
                        in_=qa[:, :, parity * P:(parity + 1) * P])

            def emit_ag(w):
                nc.gpsimd.collective_compute(
                    "AllGather", mybir.AluOpType.bypass,
                    replica_groups=[list(range(cfg.n_cores))],
                    ins=[q_locs[w].opt()],
                    outs=[q_alls[w].opt()])

            xq_full = xq_d.ap()                              # [npair, 512]
            xq_x = xq_d.ap()[:, 0:2 * P]                     # x halves
            xq_q = xq_d.ap()[:, 2 * P:4 * P]                 # q halves

            def tile_pass1(t, ge, ge0, gl, gl0, pp, qrow, is_last):
                ce = int(BeR[t]) // P
                cl = int(BlR[t]) // P
                ne = int(Be[t])
                Ke = int(Be[t]) - P * (ce - 1) if is_last and ce else P
                Kl = int(Bl[t]) - P * (cl - 1) if is_last and cl else P
                ps1 = pp.tile([P, P], F32, tag="ps1", name=f"ps1_{t}")
                ps2 = None
                if ne > 0:
                    ps2 = pp.tile([P, P], F32, tag="ps2", name=f"ps2_{t}")
                # early chunks: full pair rows [x0|x1|q0|q1]
                for j in range(ce):
                    gc = (int(ebase[t])) // P + j
                    lc = (int(ebase[t]) - ge0) // P + j
                    K = Ke if j == ce - 1 else P
                    oh = onehot(dloc1, gc)
                    nc.tensor.matmul(ps1[:], lhsT=oh[0:K, 0:P],
                                     rhs=ge[0:K, lc * 4 * P:lc * 4 * P + P],
                                     start=(j == 0), stop=False)
                    nc.tensor.matmul(ps1[:], lhsT=oh[0:K, P:2 * P],
                                     rhs=ge[0:K, lc * 4 * P + P:lc * 4 * P + 2 * P],
                                     start=False, stop=False)
                    nc.tensor.matmul(ps2[:], lhsT=oh[0:K, 0:P],
                                     rhs=ge[0:K, lc * 4 * P + 2 * P:
                                           lc * 4 * P + 3 * P],
                                     start=(j == 0), stop=False)
                    nc.tensor.matmul(ps2[:], lhsT=oh[0:K, P:2 * P],
                                     rhs=ge[0:K, lc * 4 * P + 3 * P:
                                           lc * 4 * P + 4 * P],
                                     start=False, stop=(j == ce - 1))
                # late chunks: x halves only [x0|x1]
                for j in range(cl):
                    gc = (int(lbase[t])) // P + j
                    lc = (int(lbase[t]) - gl0) // P + j
                    K = Kl if j == cl - 1 else P
                    oh = onehot(dloc1, gc)
                    nc.tensor.matmul(ps1[:], lhsT=oh[0:K, 0:P],
                                     rhs=gl[0:K, lc * 2 * P:lc * 2 * P + P],
                                     start=(ce == 0 and j == 0), stop=False)
                    nc.tensor.matmul(ps1[:], lhsT=oh[0:K, P:2 * P],
                                     rhs=gl[0:K, lc * 2 * P + P:lc * 2 * P + 2 * P],
                                     start=False, stop=(j == cl - 1))
                agg_nm = sp.tile([P, P], BF16, tag="aggnm")
                nc.scalar.activation(agg_nm[:], ps1[:], AF.Copy,
                                     scale=dinv[:, t:t + 1])
                ps_t = pp.tile([P, P], BF16, tag="pst", name=f"pst{t}", bufs=1)
                nc.tensor.transpose(ps_t[:], agg_nm[:], ident[:])
                aggrT = sp.tile([P, P], BF16, tag="aggrT")
                nc.scalar.activation(aggrT[:], ps_t[:], AF.Copy)
                for h in range(2):
                    ps_h = pp.tile([P, P], F32, tag="psh", name=f"psh{t}_{h}")
                    nc.tensor.matmul(ps_h[:], lhsT=w1n[:, h * P:(h + 1) * P],
                                     rhs=aggrT[:], start=True, stop=False)
                    nc.tensor.matmul(ps_h[:], lhsT=w1r[:, h * P:(h + 1) * P],
                                     rhs=xt[:, t * P:(t + 1) * P],
                                     start=False, stop=True)
                    nc.scalar.activation(
                        hT[:, h * npad + t * P:h * npad + (t + 1) * P],
                        ps_h[:], AF.Relu, bias=b1c[:, h:h + 1])
                ps_q = pp.tile([P, P], F32, tag="psq", name=f"psq{t}", bufs=1)
                nc.tensor.matmul(ps_q[:], lhsT=hT[:, t * P:(t + 1) * P],
                                 rhs=w2n[:, 0:P], start=True, stop=False)
                nc.tensor.matmul(ps_q[:],
                                 lhsT=hT[:, npad + t * P:npad + (t + 1) * P],
                                 rhs=w2n[:, P:2 * P], start=False, stop=True)
                qsb = sp.tile([P, P], BF16, tag="qsb")
                nc.scalar.activation(qsb[:], ps_q[:], AF.Copy)
                w = int(cfg.wave_of[t])
                nc.scalar.dma_start(out=q_locs[w][qrow:qrow + P, :], in_=qsb[:])
                if ne > 0:
                    nc.scalar.activation(stash[:, t * P:(t + 1) * P],
                                         ps2[:], AF.Copy)

            # ------------------- pass 1: waves -------------------
            nc.gpsimd.collective_compute(
                "AllGather", mybir.AluOpType.bypass,
                replica_groups=[list(range(cfg.n_cores))],
                ins=[warm_in.opt()], outs=[warm_out.opt()])
            with (tc.tile_pool(name="gath", bufs=3) as gp,
                  tc.tile_pool(name="ps1p", bufs=2, space="PSUM") as pp1):
                gidx = 0
                for w in range(W):
                    first = True
                    for grp in [g for g in cfg.groups
                                if cfg.wave_of[g[0]] == w]:
                        e0, esz, l0, lsz = spans[gidx]
                        gidx += 1
                        gb = gp.tile([P, max_g1], BF16, tag="g")
                        ge = gb[:, :esz * 4] if esz else None
                        gl = gb[:, esz * 4:esz * 4 + lsz * 2]
                        if esz:
                            nc.gpsimd.dma_gather(
                                ge.rearrange("p (c e) -> p c e", e=4 * P),
                                xq_full, eidx1[:, e0 // 16:(e0 + esz) // 16],
                                esz, esz, 4 * P, single_packet=False)
                        nc.gpsimd.dma_gather(
                            gl.rearrange("p (c e) -> p c e", e=2 * P),
                            xq_x, eidx1[:, l0 // 16:(l0 + lsz) // 16],
                            lsz, lsz, 2 * P,
                            elem_step=4 * P, single_packet=False)
                        for t in grp:
                            wt0 = int(cfg.wave_tiles[w][0])
                            tile_pass1(int(t), ge, e0, gl, l0, pp1,
                                       (int(t) - wt0) * P,
                                       int(t) == int(grp[-1]))
                        if first and w >= 1:
                            emit_ag(w - 1)
                            if w >= 2:
                                emit_repack(w - 2)
                        first = False
                emit_ag(W - 1)
                emit_repack(W - 2)
                emit_repack(W - 1)

            # ------------------- pass 2 -------------------
            with (tc.tile_pool(name="gath2", bufs=4) as gp2,
                  tc.tile_pool(name="ps2p", bufs=2, space="PSUM") as pp2):
                for grp in g2sizes:
                    s0 = int(base2[grp[0]])
                    nidx = int(sum(B2[u] for u in grp))
                    g2 = gp2.tile([P, max_g2], BF16, tag="g2")
                    nc.gpsimd.dma_gather(
                        g2[:, :nidx * 2].rearrange("p (c e) -> p c e", e=2 * P),
                        xq_q, eidx2[:, s0 // 16:(s0 + nidx) // 16],
                        nidx, nidx, 2 * P,
                        elem_step=4 * P, single_packet=False)
                    for t in grp:
                        ct = int(B2[t]) // P
                        K2l = int(Bl[t]) - P * (ct - 1) \
                            if t == grp[-1] else P
                        ps_b = pp2.tile([P, P], F32, tag="psb", name=f"psb{t}")
                        first = True
                        if Be[t] > 0:
                            nc.tensor.matmul(ps_b[:], lhsT=ident[:],
                                             rhs=stash[:, t * P:(t + 1) * P],
                                             start=True, stop=False)
                            first = False
                        for j in range(ct):
                            gc = int(base2[t]) // P + j
                            lc = (int(base2[t]) - s0) // P + j
                            K = K2l if j == ct - 1 else P
                            oh = onehot(dloc2, gc)
                            nc.tensor.matmul(
                                ps_b[:], lhsT=oh[0:K, 0:P],
                                rhs=g2[0:K, lc * 2 * P:lc * 2 * P + P],
                                start=first, stop=False)
                            first = False
                            nc.tensor.matmul(
                                ps_b[:], lhsT=oh[0:K, P:2 * P],
                                rhs=g2[0:K, lc * 2 * P + P:lc * 2 * P + 2 * P],
                                start=False, stop=(j == ct - 1))
                        agg_sb = sp.tile([P, P], F32, tag="aggsb")
                        nc.scalar.activation(agg_sb[:], ps_b[:], AF.Copy,
                                             scale=dinv[:, t:t + 1])
                        ps_r = pp2.tile([P, P], F32, tag="psr", name=f"psr{t}")
                        nc.tensor.matmul(ps_r[:], lhsT=hT[:, t * P:(t + 1) * P],
                                         rhs=w2r[:, 0:P], start=True, stop=False)
                        nc.tensor.matmul(
                            ps_r[:], lhsT=hT[:, npad + t * P:npad + (t + 1) * P],
                            rhs=w2r[:, P:2 * P], start=False, stop=False)
                        nc.tensor.matmul(ps_r[:], lhsT=ones1[:], rhs=b2r[:],
                                         start=False, stop=True)
                        osb = sp.tile([P, P], F32, tag="osb")
                        nc.vector.tensor_tensor(out=osb[:], in0=agg_sb[:],
                                                in1=ps_r[:], op=OP.add)
                        nc.sync.dma_start(out=out_d.ap()[t * P:(t + 1) * P, :],
                                          in_=osb[:])

    nc.compile()
    return nc


# --------------------------------------------------------------------------
# entry point
# --------------------------------------------------------------------------

_CACHE = {}


def prepare(inputs, cfg=None):
    x = np.asarray(inputs["x"], np.float32)
    if cfg is None:
        cfg = Cfg(n=x.shape[0])
    budgets, in_maps = pack_inputs(
        cfg, x, inputs["edge_index"],
        inputs["W1_nbr"], inputs["W1_root"], inputs["b1"],
        inputs["W2_nbr"], inputs["W2_root"], inputs["b2"])
    key = (cfg.key(), budgets)
    nc = _CACHE.get(key)
    if nc is None:
        nc = build_program(cfg, budgets)
        _CACHE[key] = nc
    return nc, in_maps, cfg


def kernel(**inputs) -> np.ndarray:
    nc, in_maps, cfg = prepare(inputs)
    res = bass_utils.run_bass_kernel_spmd(
        nc, in_maps, core_ids=list(range(cfg.n_cores)))
    out = np.concatenate([res.results[c]["out"] for c in range(cfg.n_cores)],
                         axis=0)
    return np.ascontiguousarray(out[cfg.newpos], dtype=np.float32)


# revision 5
# speedup vs baseline: 1.0076x; 1.0076x over previous
"""Trainium2 Bass kernel v2: 2-layer mean-aggregation GraphSAGE encoder.

Wave-pipelined combined gather. Nodes (relabeled by degree-balanced binning
into 8 cores x 49 tiles of 128) are processed in W waves of ~4 tiles. A
DRAM table xq holds one 1KB bf16 row per node PAIR: [x(2i)|x(2i+1)|q(2i)|
q(2i+1)]; the q halves are filled during execution as each wave's layer-1
output q = h @ W2n.T is AllGather'd and repacked in.

Per dst tile, edges are split into EARLY (src wave <= dst wave - lag, q
already available) and LATE. Pass 1 gathers, per edge slot, the full 1KB
pair-row for early edges (x half feeds the layer-1 one-hot segment-sum, q
half feeds layer-2 partial aggregation with the SAME one-hot) but only the
512B x-half for late edges. Pass 2 gathers only the late edges' 512B
q-half. SWDGE descriptor generation on the Pool engine (~8ns/descriptor,
the kernel's critical resource) thus drops from 2E to (1 + late)E per core,
and gathered bytes stay under the DMA engines' pacing threshold.

A slot's one-hot column encodes dst-in-tile + 128*parity(src), so one
[128,256] DVE is_equal per chunk yields both the even-src and odd-src
one-hots. Layer-2 partial sums are stashed unscaled in SBUF (bf16) and
added back in pass 2 via an identity matmul into PSUM before the deg_inv
scale. AllGather latency (~55us end-to-end, strictly serialized on the CC
cores) is hidden by sizing waves >= that latency and triggering each AG
mid-next-wave; repacks run two waves after their AG.

All index bookkeeping is host-side on edge_index only; feature FLOPs run on
the NeuronCores in bf16 with fp32 PSUM accumulation.
"""

import numpy as np
import ml_dtypes

import concourse.bass as bass
from concourse import bacc, mybir, tile
from concourse import bass_utils

P = 128
F32 = mybir.dt.float32
BF16 = mybir.dt.bfloat16
I16 = mybir.dt.int16
BF = ml_dtypes.bfloat16


def rup(v):
    return (-(-np.asarray(v) // P) * P).astype(np.int64)


class Cfg:
    def __init__(self, n=50000, n_cores=8, in_dim=128, hid=256, out_dim=128,
                 wave_sizes=None, lag=3):
        assert n % n_cores == 0
        self.n = n
        self.n_cores = n_cores
        self.in_dim, self.hid, self.out_dim = in_dim, hid, out_dim
        self.npc = n // n_cores
        self.nt = -(-self.npc // P)          # 49 dst tiles per core
        self.npad = self.nt * P              # 6272
        self.npos = n_cores * self.npad      # 50176
        self.npair = self.npos // 2
        if wave_sizes is None:
            wave_sizes = [4] * (self.nt // 4)
            if self.nt % 4:
                wave_sizes.append(self.nt % 4)
        assert sum(wave_sizes) == self.nt
        self.W = len(wave_sizes)
        self.lag = lag
        self.wave_of = np.repeat(np.arange(self.W), wave_sizes)
        self.wave_tiles = [np.nonzero(self.wave_of == w)[0]
                           for w in range(self.W)]
        # gather groups: consecutive tiles, <= 2 per group, within one wave
        self.groups = []
        self.tile_lag = np.full(self.nt, lag, np.int64)
        for w in range(self.W):
            wt = [int(t) for t in self.wave_tiles[w]]
            for i in range(0, len(wt), 2):
                self.groups.append(wt[i:i + 2])
                if i > 0:
                    # later groups of a wave gather after repack(w-lag+1)
                    # has landed (it is emitted between the groups), so
                    # their tiles can classify one wave less conservatively
                    for t in wt[i:i + 2]:
                        self.tile_lag[t] = lag - 1
        self.newpos = None

    def key(self):
        return (self.n, self.n_cores, self.W, self.lag)


def _rank_in_groups(seg, nseg):
    start = np.searchsorted(seg, np.arange(nseg))
    return np.arange(len(seg)) - start[seg]


def pass2_order_groups(nt, B2):
    """Tiles in descending-B2 order (smallest last, to shrink the kernel
    tail), grouped into <=6144-slot gathers; returns (order, groups,
    base2) with base2 giving each tile's slot offset in that layout."""
    order = sorted(range(nt), key=lambda t: (-int(B2[t]), t))
    groups = []
    gi = 0
    while gi < nt:
        grp = [order[gi]]
        while (gi + len(grp) < nt and len(grp) < 4
               and sum(int(B2[u]) for u in grp)
               + int(B2[order[gi + len(grp)]]) <= 6144):
            grp.append(order[gi + len(grp)])
        groups.append(grp)
        gi += len(grp)
    if len(groups[-1]) > 1:
        last = groups[-1]
        groups[-1] = last[:-1]
        groups.append(last[-1:])
    base2 = np.zeros(nt + 1, np.int64)
    off = 0
    for t in order:
        base2[t] = off
        off += int(B2[t])
    base2[nt] = off
    return order, groups, base2


def slot_layout(cfg, Be, Bl):
    """Global slot ranges for pass 1: per gather-group, early slots of its
    tiles (each tile's range rounded to 128), then late slots likewise.
    Returns (ebase[t], lbase[t], group spans, tot1)."""
    BeR, BlR = rup(Be), rup(Bl)
    ebase = np.zeros(cfg.nt, np.int64)
    lbase = np.zeros(cfg.nt, np.int64)
    spans = []          # per group: (e0, esz, l0, lsz)
    off = 0
    for grp in cfg.groups:
        e0 = off
        for t in grp:
            ebase[t] = off
            off += int(BeR[t])
        l0 = off
        for t in grp:
            lbase[t] = off
            off += int(BlR[t])
        spans.append((e0, l0 - e0, l0, off - l0))
    return ebase, lbase, spans, off


def pack_inputs(cfg, x, edge_index, w1n, w1r, b1, w2n, w2r, b2):
    x = np.asarray(x, np.float32)
    src = np.asarray(edge_index[0], np.int64)
    dst = np.asarray(edge_index[1], np.int64)
    n, nt, npad = cfg.n, cfg.nt, cfg.npad

    deg = np.bincount(dst, minlength=n).astype(np.float32)
    deginv = np.where(deg > 0,
                      np.float32(1.0) / np.maximum(deg, np.float32(1.0)),
                      np.float32(0.0)).astype(np.float32)

    # degree-balanced serpentine assignment of dst nodes to (core,tile) bins
    nbins = cfg.n_cores * nt
    order = np.argsort(-deg, kind="stable")
    r = np.arange(n)
    p_pass = r // nbins
    j = r % nbins
    b = np.where(p_pass % 2 == 0, j, nbins - 1 - j)
    newpos = np.empty(n, np.int64)
    newpos[order] = b * P + p_pass
    cfg.newpos = newpos

    spos = newpos[src]
    dpos = newpos[dst]
    core = dpos // npad
    tl = (dpos % npad) // P
    di = (dpos % P).astype(np.int64)
    par = spos & 1
    pairidx = spos >> 1
    wsrc = cfg.wave_of[(spos % npad) // P]
    wdst = cfg.wave_of[tl]
    early = wsrc <= wdst - cfg.tile_lag[tl]
    dval = (di + P * par).astype(np.float32)

    key = core * nt + tl
    cnt_e = np.bincount(key[early], minlength=cfg.n_cores * nt).reshape(
        cfg.n_cores, nt)
    cnt_l = np.bincount(key[~early], minlength=cfg.n_cores * nt).reshape(
        cfg.n_cores, nt)
    Be = cnt_e.max(0)
    Bl = np.maximum(cnt_l.max(0), 1)
    ebase, lbase, spans, tot1 = slot_layout(cfg, Be, Bl)
    B2 = rup(Bl)
    _, g2grps, base2 = pass2_order_groups(nt, B2)
    tot2 = int(base2[nt])

    slotnode = np.full(cfg.npos, -1, np.int64)
    slotnode[newpos] = np.arange(n)
    m_sl = slotnode >= 0

    # xq pair rows: [x(2i) | x(2i+1) | q(2i) | q(2i+1)]
    xq4 = np.zeros((cfg.npair, 4, P), BF)
    xn = np.zeros((cfg.npos, P), BF)
    xn[m_sl] = x[slotnode[m_sl]].astype(BF)
    xq4[:, 0, :] = xn[0::2]
    xq4[:, 1, :] = xn[1::2]
    xq = np.ascontiguousarray(xq4.reshape(cfg.npair, 4 * P))

    w1n = np.asarray(w1n, np.float32)
    w1r = np.asarray(w1r, np.float32)
    w2n = np.asarray(w2n, np.float32)
    w2r = np.asarray(w2r, np.float32)
    iota = np.tile(np.arange(2 * P, dtype=np.float32), (P, 1))
    shared = {
        "xq": xq,
        "w1nT": np.ascontiguousarray(w1n.T.astype(BF)),
        "w1rT": np.ascontiguousarray(w1r.T.astype(BF)),
        "b1c": np.ascontiguousarray(np.asarray(b1, np.float32).reshape(2, P).T),
        "w2nT": np.ascontiguousarray(
            np.concatenate([w2n.T[:P, :], w2n.T[P:, :]], axis=1).astype(BF)),
        "w2rT": np.ascontiguousarray(
            np.concatenate([w2r.T[:P, :], w2r.T[P:, :]], axis=1).astype(BF)),
        "b2r": np.ascontiguousarray(np.asarray(b2, np.float32).reshape(1, P)
                                    .astype(BF)),
        "iota2": np.ascontiguousarray(iota.astype(BF)),
        "ident": np.ascontiguousarray(np.eye(P, dtype=np.float32).astype(BF)),
    }

    in_maps = []
    for c in range(cfg.n_cores):
        m = core == c
        etl, edi = tl[m], dval[m]
        epair, eearly = pairidx[m], early[m]

        so = np.argsort(etl * 2 + (~eearly).astype(np.int64), kind="stable")
        s_tl, s_early = etl[so], eearly[so]
        seg = s_tl * 2 + (~s_early).astype(np.int64)
        rank = _rank_in_groups(seg, 2 * nt)
        slot1 = np.where(s_early, ebase[s_tl] + rank, lbase[s_tl] + rank)
        eidx1 = np.zeros(tot1, np.int64)
        dloc1 = np.full(tot1, -1.0, np.float32)
        BeR_, BlR_ = rup(Be), rup(Bl)
        for grp in cfg.groups:
            tl_ = grp[-1]
            eidx1[ebase[tl_] + Be[tl_]:ebase[tl_] + BeR_[tl_]] = -1
            eidx1[lbase[tl_] + Bl[tl_]:lbase[tl_] + BlR_[tl_]] = -1
        eidx1[slot1] = epair[so]
        dloc1[slot1] = edi[so]

        lm = ~s_early
        slot2 = base2[s_tl[lm]] + rank[lm]
        eidx2 = np.zeros(tot2, np.int64)
        dloc2 = np.full(tot2, -1.0, np.float32)
        for grp in g2grps:
            tl_ = grp[-1]
            eidx2[base2[tl_] + Bl[tl_]:base2[tl_] + B2[tl_]] = -1
        eidx2[slot2] = epair[so][lm]
        dloc2[slot2] = edi[so][lm]

        sl = slotnode[c * npad:(c + 1) * npad]
        msl = sl >= 0
        xt = np.zeros((P, npad), BF)
        xt[:, msl] = x[sl[msl]].T.astype(BF)
        dinv = np.zeros((P, nt), np.float32)
        dinv.T.flat[msl] = deginv[sl[msl]]

        im = dict(shared)
        im["eidx1"] = np.ascontiguousarray(
            np.tile(eidx1.astype(np.int16).reshape(-1, 16).T, (8, 1)))
        im["dloc1"] = np.ascontiguousarray(dloc1.reshape(-1, P).T.astype(BF))
        im["eidx2"] = np.ascontiguousarray(
            np.tile(eidx2.astype(np.int16).reshape(-1, 16).T, (8, 1)))
        im["dloc2"] = np.ascontiguousarray(dloc2.reshape(-1, P).T.astype(BF))
        im["xtile"] = xt
        im["dinv"] = dinv
        in_maps.append(im)

    budgets = (tuple(int(v) for v in Be), tuple(int(v) for v in Bl))
    return budgets, in_maps


# --------------------------------------------------------------------------
# Bass program
# --------------------------------------------------------------------------

def build_program(cfg, budgets):
    Be = np.array(budgets[0], np.int64)
    Bl = np.array(budgets[1], np.int64)
    BeR, BlR = rup(Be), rup(Bl)
    ebase, lbase, spans, tot1 = slot_layout(cfg, Be, Bl)
    B2 = rup(Bl)
    nt, npad, W = cfg.nt, cfg.npad, cfg.W
    _, _g2, base2 = pass2_order_groups(nt, B2)
    tot2 = int(base2[nt])

    nc = bacc.Bacc("TRN2", target_bir_lowering=False, debug=False,
                   enable_asserts=False, num_devices=cfg.n_cores)

    xq_d = nc.dram_tensor("xq", [cfg.npair, 4 * P], BF16, kind="ExternalInput")
    e1_d = nc.dram_tensor("eidx1", [P, tot1 // 16], I16, kind="ExternalInput")
    d1_d = nc.dram_tensor("dloc1", [P, tot1 // P], BF16, kind="ExternalInput")
    e2_d = nc.dram_tensor("eidx2", [P, tot2 // 16], I16, kind="ExternalInput")
    d2_d = nc.dram_tensor("dloc2", [P, tot2 // P], BF16, kind="ExternalInput")
    xt_d = nc.dram_tensor("xtile", [P, npad], BF16, kind="ExternalInput")
    dinv_d = nc.dram_tensor("dinv", [P, nt], F32, kind="ExternalInput")
    w1n_d = nc.dram_tensor("w1nT", [P, cfg.hid], BF16, kind="ExternalInput")
    w1r_d = nc.dram_tensor("w1rT", [P, cfg.hid], BF16, kind="ExternalInput")
    b1_d = nc.dram_tensor("b1c", [P, 2], F32, kind="ExternalInput")
    w2n_d = nc.dram_tensor("w2nT", [P, 2 * P], BF16, kind="ExternalInput")
    w2r_d = nc.dram_tensor("w2rT", [P, 2 * P], BF16, kind="ExternalInput")
    b2_d = nc.dram_tensor("b2r", [1, P], BF16, kind="ExternalInput")
    iota_d = nc.dram_tensor("iota2", [P, 2 * P], BF16, kind="ExternalInput")
    id_d = nc.dram_tensor("ident", [P, P], BF16, kind="ExternalInput")
    out_d = nc.dram_tensor("out", [npad, P], F32, kind="ExternalOutput")

    AF = mybir.ActivationFunctionType
    OP = mybir.AluOpType

    # max gather-buffer size (bf16 elems per partition): early rows (4P
    # elems each) plus late x-halves (2P) share one pooled buffer
    max_g1 = max(es * 4 + ls * 2 for (_, es, _, ls) in spans)
    _, g2sizes, base2 = pass2_order_groups(nt, B2)
    max_g2 = max(sum(int(B2[u]) for u in grp) for grp in g2sizes) * 2

    with tile.TileContext(nc) as tc:
        with (tc.tile_pool(name="const", bufs=1) as cp,
              tc.tile_pool(name="dram", bufs=1, space="DRAM") as dp,
              tc.tile_pool(name="oh", bufs=6) as ohp,
              tc.tile_pool(name="stage", bufs=4) as sp):
            eidx1 = cp.tile([P, tot1 // 16], I16)
            nc.sync.dma_start(out=eidx1[:], in_=e1_d.ap())
            dloc1 = cp.tile([P, tot1 // P], BF16)
            nc.sync.dma_start(out=dloc1[:], in_=d1_d.ap())
            iota2 = cp.tile([P, 2 * P], BF16)
            nc.sync.dma_start(out=iota2[:], in_=iota_d.ap())
            ident = cp.tile([P, P], BF16)
            nc.sync.dma_start(out=ident[:], in_=id_d.ap())
            xt = cp.tile([P, npad], BF16)
            nc.sync.dma_start(out=xt[:], in_=xt_d.ap())
            dinv = cp.tile([P, nt], F32)
            nc.sync.dma_start(out=dinv[:], in_=dinv_d.ap())
            w1n = cp.tile([P, cfg.hid], BF16)
            nc.sync.dma_start(out=w1n[:], in_=w1n_d.ap())
            w1r = cp.tile([P, cfg.hid], BF16)
            nc.sync.dma_start(out=w1r[:], in_=w1r_d.ap())
            b1c = cp.tile([P, 2], F32)
            nc.sync.dma_start(out=b1c[:], in_=b1_d.ap())
            w2n = cp.tile([P, 2 * P], BF16)
            nc.sync.dma_start(out=w2n[:], in_=w2n_d.ap())
            w2r = cp.tile([P, 2 * P], BF16)
            nc.sync.dma_start(out=w2r[:], in_=w2r_d.ap())
            b2r = cp.tile([1, P], BF16)
            nc.sync.dma_start(out=b2r[:], in_=b2_d.ap())
            eidx2 = cp.tile([P, tot2 // 16], I16)
            nc.sync.dma_start(out=eidx2[:], in_=e2_d.ap())
            dloc2 = cp.tile([P, tot2 // P], BF16)
            nc.sync.dma_start(out=dloc2[:], in_=d2_d.ap())
            ones1 = cp.tile([1, P], BF16)
            nc.vector.memset(ones1[:], 1.0)
            hT = cp.tile([P, 2 * npad], BF16)
            stash = cp.tile([P, nt * P], BF16)

            warm_in = dp.tile([P, P], BF16, name="warm_in")
            warm_out = dp.tile([cfg.n_cores * P, P], BF16, name="warm_out",
                               addr_space="Shared")
            q_locs, q_alls = [], []
            for w in range(W):
                rows = len(cfg.wave_tiles[w]) * P
                q_locs.append(dp.tile([rows, P], BF16, name=f"qloc{w}"))
                q_alls.append(dp.tile([cfg.n_cores * rows, P], BF16,
                                      name=f"qall{w}", addr_space="Shared"))

            def onehot(dloc_t, gc):
                oh = ohp.tile([P, 2 * P], BF16, tag="oh")
                nc.vector.tensor_tensor(
                    out=oh[:], in0=iota2[:],
                    in1=dloc_t[:, gc:gc + 1].to_broadcast([P, 2 * P]),
                    op=OP.is_equal)
                return oh

            def emit_repack(w):
                """q_all[w] rows -> q-halves of xq pair rows (both parities).
                q(pos) lives at pair pos>>1, elem offset (2+(pos&1))*128."""
                rows = len(cfg.wave_tiles[w]) * P
                t0 = int(cfg.wave_tiles[w][0])
                pr0 = t0 * P // 2
                npr = rows // 2
                qa = q_alls[w][:, :].rearrange("(c j t) e -> c j (t e)",
                                               c=cfg.n_cores, t=2)
                xqv = xq_d.ap().rearrange("(c q) e -> c q e", c=cfg.n_cores)
                for parity in range(2):
                    nc.sync.dma_start(
                        out=xqv[:, pr0:pr0 + npr,
                                (2 + parity) * P:(3 + parity) * P],
                        in_=qa[:, :, parity * P:(parity + 1) * P])

            def emit_ag(w):
                nc.gpsimd.collective_compute(
                    "AllGather", mybir.AluOpType.bypass,
                    replica_groups=[list(range(cfg.n_cores))],
                    ins=[q_locs[w].opt()],
                    outs=[q_alls[w].opt()])

            xq_full = xq_d.ap()                              # [npair, 512]
            xq_x = xq_d.ap()[:, 0:2 * P]                     # x halves
            xq_q = xq_d.ap()[:, 2 * P:4 * P]                 # q halves

            def tile_pass1(t, ge, ge0, gl, gl0, pp, qrow, is_last):
                ce = int(BeR[t]) // P
                cl = int(BlR[t]) // P
                ne = int(Be[t])
                Ke = int(Be[t]) - P * (ce - 1) if is_last and ce else P
                Kl = int(Bl[t]) - P * (cl - 1) if is_last and cl else P
                ps1 = pp.tile([P, P], F32, tag="ps1", name=f"ps1_{t}")
                ps2 = None
                if ne > 0:
                    ps2 = pp.tile([P, P], F32, tag="ps2", name=f"ps2_{t}")
                # early chunks: full pair rows [x0|x1|q0|q1]
                for j in range(ce):
                    gc = (int(ebase[t])) // P + j
                    lc = (int(ebase[t]) - ge0) // P + j
                    K = Ke if j == ce - 1 else P
                    oh = onehot(dloc1, gc)
                    nc.tensor.matmul(ps1[:], lhsT=oh[0:K, 0:P],
                                     rhs=ge[0:K, lc * 4 * P:lc * 4 * P + P],
                                     start=(j == 0), stop=False)
                    nc.tensor.matmul(ps1[:], lhsT=oh[0:K, P:2 * P],
                                     rhs=ge[0:K, lc * 4 * P + P:lc * 4 * P + 2 * P],
                                     start=False, stop=False)
                    nc.tensor.matmul(ps2[:], lhsT=oh[0:K, 0:P],
                                     rhs=ge[0:K, lc * 4 * P + 2 * P:
                                           lc * 4 * P + 3 * P],
                                     start=(j == 0), stop=False)
                    nc.tensor.matmul(ps2[:], lhsT=oh[0:K, P:2 * P],
                                     rhs=ge[0:K, lc * 4 * P + 3 * P:
                                           lc * 4 * P + 4 * P],
                                     start=False, stop=(j == ce - 1))
                # late chunks: x halves only [x0|x1]
                for j in range(cl):
                    gc = (int(lbase[t])) // P + j
                    lc = (int(lbase[t]) - gl0) // P + j
                    K = Kl if j == cl - 1 else P
                    oh = onehot(dloc1, gc)
                    nc.tensor.matmul(ps1[:], lhsT=oh[0:K, 0:P],
                                     rhs=gl[0:K, lc * 2 * P:lc * 2 * P + P],
                                     start=(ce == 0 and j == 0), stop=False)
                    nc.tensor.matmul(ps1[:], lhsT=oh[0:K, P:2 * P],
                                     rhs=gl[0:K, lc * 2 * P + P:lc * 2 * P + 2 * P],
                                     start=False, stop=(j == cl - 1))
                agg_nm = sp.tile([P, P], BF16, tag="aggnm")
                nc.scalar.activation(agg_nm[:], ps1[:], AF.Copy,
                                     scale=dinv[:, t:t + 1])
                ps_t = pp.tile([P, P], BF16, tag="pst", name=f"pst{t}", bufs=1)
                nc.tensor.transpose(ps_t[:], agg_nm[:], ident[:])
                aggrT = sp.tile([P, P], BF16, tag="aggrT")
                nc.scalar.activation(aggrT[:], ps_t[:], AF.Copy)
                for h in range(2):
                    ps_h = pp.tile([P, P], F32, tag="psh", name=f"psh{t}_{h}")
                    nc.tensor.matmul(ps_h[:], lhsT=w1n[:, h * P:(h + 1) * P],
                                     rhs=aggrT[:], start=True, stop=False)
                    nc.tensor.matmul(ps_h[:], lhsT=w1r[:, h * P:(h + 1) * P],
                                     rhs=xt[:, t * P:(t + 1) * P],
                                     start=False, stop=True)
                    nc.scalar.activation(
                        hT[:, h * npad + t * P:h * npad + (t + 1) * P],
                        ps_h[:], AF.Relu, bias=b1c[:, h:h + 1])
                ps_q = pp.tile([P, P], F32, tag="psq", name=f"psq{t}", bufs=1)
                nc.tensor.matmul(ps_q[:], lhsT=hT[:, t * P:(t + 1) * P],
                                 rhs=w2n[:, 0:P], start=True, stop=False)
                nc.tensor.matmul(ps_q[:],
                                 lhsT=hT[:, npad + t * P:npad + (t + 1) * P],
                                 rhs=w2n[:, P:2 * P], start=False, stop=True)
                qsb = sp.tile([P, P], BF16, tag="qsb")
                nc.scalar.activation(qsb[:], ps_q[:], AF.Copy)
                w = int(cfg.wave_of[t])
                nc.scalar.dma_start(out=q_locs[w][qrow:qrow + P, :], in_=qsb[:])
                if ne > 0:
                    nc.scalar.activation(stash[:, t * P:(t + 1) * P],
                                         ps2[:], AF.Copy)

            # ------------------- pass 1: waves -------------------
            nc.gpsimd.collective_compute(
                "AllGather", mybir.AluOpType.bypass,
                replica_groups=[list(range(cfg.n_cores))],
                ins=[warm_in.opt()], outs=[warm_out.opt()])
            with (tc.tile_pool(name="gath", bufs=3) as gp,
                  tc.tile_pool(name="ps1p", bufs=2, space="PSUM") as pp1):
                gidx = 0
                for w in range(W):
                    first = True
                    for grp in [g for g in cfg.groups
                                if cfg.wave_of[g[0]] == w]:
                        e0, esz, l0, lsz = spans[gidx]
                        gidx += 1
                        gb = gp.tile([P, max_g1], BF16, tag="g")
                        ge = gb[:, :esz * 4] if esz else None
                        gl = gb[:, esz * 4:esz * 4 + lsz * 2]
                        if esz:
                            nc.gpsimd.dma_gather(
                                ge.rearrange("p (c e) -> p c e", e=4 * P),
                                xq_full, eidx1[:, e0 // 16:(e0 + esz) // 16],
                                esz, esz, 4 * P, single_packet=False)
                        nc.gpsimd.dma_gather(
                            gl.rearrange("p (c e) -> p c e", e=2 * P),
                            xq_x, eidx1[:, l0 // 16:(l0 + lsz) // 16],
                            lsz, lsz, 2 * P,
                            elem_step=4 * P, single_packet=False)
                        for t in grp:
                            wt0 = int(cfg.wave_tiles[w][0])
                            tile_pass1(int(t), ge, e0, gl, l0, pp1,
                                       (int(t) - wt0) * P,
                                       int(t) == int(grp[-1]))
                        if first and w >= 1:
                            emit_ag(w - 1)
                            if w >= 2:
                                emit_repack(w - 2)
                        first = False
                emit_ag(W - 1)
                emit_repack(W - 2)
                emit_repack(W - 1)

            # ------------------- pass 2 -------------------
            with (tc.tile_pool(name="gath2", bufs=4) as gp2,
                  tc.tile_pool(name="ps2p", bufs=2, space="PSUM") as pp2):
                for grp in g2sizes:
                    s0 = int(base2[grp[0]])
                    nidx = int(sum(B2[u] for u in grp))
                    g2 = gp2.tile([P, max_g2], BF16, tag="g2")
                    nc.gpsimd.dma_gather(
                        g2[:, :nidx * 2].rearrange("p (c e) -> p c e", e=2 * P),
                        xq_q, eidx2[:, s0 // 16:(s0 + nidx) // 16],
                        nidx, nidx, 2 * P,
                        elem_step=4 * P, single_packet=False)
                    for t in grp:
                        ct = int(B2[t]) // P
                        K2l = int(Bl[t]) - P * (ct - 1) \
                            if t == grp[-1] else P
                        ps_b = pp2.tile([P, P], F32, tag="psb", name=f"psb{t}")
                        first = True
                        if Be[t] > 0:
                            nc.tensor.matmul(ps_b[:], lhsT=ident[:],
                                             rhs=stash[:, t * P:(t + 1) * P],
                                             start=True, stop=False)
                            first = False
                        for j in range(ct):
                            gc = int(base2[t]) // P + j
                            lc = (int(base2[t]) - s0) // P + j
                            K = K2l if j == ct - 1 else P
                            oh = onehot(dloc2, gc)
                            nc.tensor.matmul(
                                ps_b[:], lhsT=oh[0:K, 0:P],
                                rhs=g2[0:K, lc * 2 * P:lc * 2 * P + P],
                                start=first, stop=False)
                            first = False
                            nc.tensor.matmul(
                                ps_b[:], lhsT=oh[0:K, P:2 * P],
                                rhs=g2[0:K, lc * 2 * P + P:lc * 2 * P + 2 * P],
                                start=False, stop=(j == ct - 1))
                        agg_sb = sp.tile([P, P], F32, tag="aggsb")
                        nc.scalar.activation(agg_sb[:], ps_b[:], AF.Copy,
                                             scale=dinv[:, t:t + 1])
                        ps_r = pp2.tile([P, P], F32, tag="psr", name=f"psr{t}")
                        nc.tensor.matmul(ps_r[:], lhsT=hT[:, t * P:(t + 1) * P],
                                         rhs=w2r[:, 0:P], start=True, stop=False)
                        nc.tensor.matmul(
                            ps_r[:], lhsT=hT[:, npad + t * P:npad + (t + 1) * P],
                            rhs=w2r[:, P:2 * P], start=False, stop=False)
                        nc.tensor.matmul(ps_r[:], lhsT=ones1[:], rhs=b2r[:],
                                         start=False, stop=True)
                        osb = sp.tile([P, P], F32, tag="osb")
                        nc.vector.tensor_tensor(out=osb[:], in0=agg_sb[:],
                                                in1=ps_r[:], op=OP.add)
                        nc.sync.dma_start(out=out_d.ap()[t * P:(t + 1) * P, :],
                                          in_=osb[:])

    nc.compile()
    return nc


# --------------------------------------------------------------------------
# entry point
# --------------------------------------------------------------------------

_CACHE = {}


def prepare(inputs, cfg=None):
    x = np.asarray(inputs["x"], np.float32)
    if cfg is None:
        cfg = Cfg(n=x.shape[0])
    budgets, in_maps = pack_inputs(
        cfg, x, inputs["edge_index"],
        inputs["W1_nbr"], inputs["W1_root"], inputs["b1"],
        inputs["W2_nbr"], inputs["W2_root"], inputs["b2"])
    key = (cfg.key(), budgets)
    nc = _CACHE.get(key)
    if nc is None:
        nc = build_program(cfg, budgets)
        _CACHE[key] = nc
    return nc, in_maps, cfg


def kernel(**inputs) -> np.ndarray:
    nc, in_maps, cfg = prepare(inputs)
    res = bass_utils.run_bass_kernel_spmd(
        nc, in_maps, core_ids=list(range(cfg.n_cores)))
    out = np.concatenate([res.results[c]["out"] for c in range(cfg.n_cores)],
                         axis=0)
    return np.ascontiguousarray(out[cfg.newpos], dtype=np.float32)


# revision 6
# speedup vs baseline: 1.0133x; 1.0057x over previous
"""Trainium2 Bass kernel v2: 2-layer mean-aggregation GraphSAGE encoder.

Wave-pipelined combined gather. Nodes (relabeled by degree-balanced binning
into 8 cores x 49 tiles of 128) are processed in W waves of ~4 tiles. A
DRAM table xq holds one 1KB bf16 row per node PAIR: [x(2i)|x(2i+1)|q(2i)|
q(2i+1)]; the q halves are filled during execution as each wave's layer-1
output q = h @ W2n.T is AllGather'd and repacked in.

Per dst tile, edges are split into EARLY (src wave <= dst wave - lag, q
already available) and LATE. Pass 1 gathers, per edge slot, the full 1KB
pair-row for early edges (x half feeds the layer-1 one-hot segment-sum, q
half feeds layer-2 partial aggregation with the SAME one-hot) but only the
512B x-half for late edges. Pass 2 gathers only the late edges' 512B
q-half. SWDGE descriptor generation on the Pool engine (~8ns/descriptor,
the kernel's critical resource) thus drops from 2E to (1 + late)E per core,
and gathered bytes stay under the DMA engines' pacing threshold.

A slot's one-hot column encodes dst-in-tile + 128*parity(src), so one
[128,256] DVE is_equal per chunk yields both the even-src and odd-src
one-hots. Layer-2 partial sums are stashed unscaled in SBUF (bf16) and
added back in pass 2 via an identity matmul into PSUM before the deg_inv
scale. AllGather latency (~55us end-to-end, strictly serialized on the CC
cores) is hidden by sizing waves >= that latency and triggering each AG
mid-next-wave; repacks run two waves after their AG.

All index bookkeeping is host-side on edge_index only; feature FLOPs run on
the NeuronCores in bf16 with fp32 PSUM accumulation.
"""

import numpy as np
import ml_dtypes

import concourse.bass as bass
from concourse import bacc, mybir, tile
from concourse import bass_utils

P = 128
F32 = mybir.dt.float32
BF16 = mybir.dt.bfloat16
I16 = mybir.dt.int16
BF = ml_dtypes.bfloat16


def rup(v):
    return (-(-np.asarray(v) // P) * P).astype(np.int64)


class Cfg:
    def __init__(self, n=50000, n_cores=8, in_dim=128, hid=256, out_dim=128,
                 wave_sizes=None, lag=3):
        assert n % n_cores == 0
        self.n = n
        self.n_cores = n_cores
        self.in_dim, self.hid, self.out_dim = in_dim, hid, out_dim
        self.npc = n // n_cores
        self.nt = -(-self.npc // P)          # 49 dst tiles per core
        self.npad = self.nt * P              # 6272
        self.npos = n_cores * self.npad      # 50176
        self.npair = self.npos // 2
        if wave_sizes is None:
            wave_sizes = [4] * (self.nt // 4)
            if self.nt % 4:
                wave_sizes[-1] += self.nt % 4
        assert sum(wave_sizes) == self.nt
        self.W = len(wave_sizes)
        self.lag = lag
        self.wave_of = np.repeat(np.arange(self.W), wave_sizes)
        self.wave_tiles = [np.nonzero(self.wave_of == w)[0]
                           for w in range(self.W)]
        # gather groups: consecutive tiles, <= 2 per group, within one wave
        self.groups = []
        self.tile_lag = np.full(self.nt, lag, np.int64)
        for w in range(self.W):
            wt = [int(t) for t in self.wave_tiles[w]]
            for i in range(0, len(wt), 2):
                self.groups.append(wt[i:i + 2])
                if i > 0:
                    # later groups of a wave gather after repack(w-lag+1)
                    # has landed (it is emitted between the groups), so
                    # their tiles can classify one wave less conservatively
                    for t in wt[i:i + 2]:
                        self.tile_lag[t] = lag - 1
        self.newpos = None

    def key(self):
        return (self.n, self.n_cores, self.W, self.lag)


def _rank_in_groups(seg, nseg):
    start = np.searchsorted(seg, np.arange(nseg))
    return np.arange(len(seg)) - start[seg]


def pass2_order_groups(nt, B2):
    """Tiles in descending-B2 order (smallest last, to shrink the kernel
    tail), grouped into <=6144-slot gathers; returns (order, groups,
    base2) with base2 giving each tile's slot offset in that layout."""
    order = sorted(range(nt), key=lambda t: (-int(B2[t]), t))
    groups = []
    gi = 0
    while gi < nt:
        grp = [order[gi]]
        while (gi + len(grp) < nt and len(grp) < 4
               and sum(int(B2[u]) for u in grp)
               + int(B2[order[gi + len(grp)]]) <= 6144):
            grp.append(order[gi + len(grp)])
        groups.append(grp)
        gi += len(grp)
    if len(groups[-1]) > 1:
        last = groups[-1]
        groups[-1] = last[:-1]
        groups.append(last[-1:])
    base2 = np.zeros(nt + 1, np.int64)
    off = 0
    for t in order:
        base2[t] = off
        off += int(B2[t])
    base2[nt] = off
    return order, groups, base2


def slot_layout(cfg, Be, Bl):
    """Global slot ranges for pass 1: per gather-group, early slots of its
    tiles (each tile's range rounded to 128), then late slots likewise.
    Returns (ebase[t], lbase[t], group spans, tot1)."""
    BeR, BlR = rup(Be), rup(Bl)
    ebase = np.zeros(cfg.nt, np.int64)
    lbase = np.zeros(cfg.nt, np.int64)
    spans = []          # per group: (e0, esz, l0, lsz)
    off = 0
    for grp in cfg.groups:
        e0 = off
        for t in grp:
            ebase[t] = off
            off += int(BeR[t])
        l0 = off
        for t in grp:
            lbase[t] = off
            off += int(BlR[t])
        spans.append((e0, l0 - e0, l0, off - l0))
    return ebase, lbase, spans, off


def pack_inputs(cfg, x, edge_index, w1n, w1r, b1, w2n, w2r, b2):
    x = np.asarray(x, np.float32)
    src = np.asarray(edge_index[0], np.int64)
    dst = np.asarray(edge_index[1], np.int64)
    n, nt, npad = cfg.n, cfg.nt, cfg.npad

    deg = np.bincount(dst, minlength=n).astype(np.float32)
    deginv = np.where(deg > 0,
                      np.float32(1.0) / np.maximum(deg, np.float32(1.0)),
                      np.float32(0.0)).astype(np.float32)

    # degree-balanced serpentine assignment of dst nodes to (core,tile) bins
    nbins = cfg.n_cores * nt
    order = np.argsort(-deg, kind="stable")
    r = np.arange(n)
    p_pass = r // nbins
    j = r % nbins
    b = np.where(p_pass % 2 == 0, j, nbins - 1 - j)
    newpos = np.empty(n, np.int64)
    newpos[order] = b * P + p_pass
    cfg.newpos = newpos

    spos = newpos[src]
    dpos = newpos[dst]
    core = dpos // npad
    tl = (dpos % npad) // P
    di = (dpos % P).astype(np.int64)
    par = spos & 1
    pairidx = spos >> 1
    wsrc = cfg.wave_of[(spos % npad) // P]
    wdst = cfg.wave_of[tl]
    early = wsrc <= wdst - cfg.tile_lag[tl]
    dval = (di + P * par).astype(np.float32)

    key = core * nt + tl
    cnt_e = np.bincount(key[early], minlength=cfg.n_cores * nt).reshape(
        cfg.n_cores, nt)
    cnt_l = np.bincount(key[~early], minlength=cfg.n_cores * nt).reshape(
        cfg.n_cores, nt)
    Be = cnt_e.max(0)
    Bl = np.maximum(cnt_l.max(0), 1)
    ebase, lbase, spans, tot1 = slot_layout(cfg, Be, Bl)
    B2 = rup(Bl)
    _, g2grps, base2 = pass2_order_groups(nt, B2)
    tot2 = int(base2[nt])

    slotnode = np.full(cfg.npos, -1, np.int64)
    slotnode[newpos] = np.arange(n)
    m_sl = slotnode >= 0

    # xq pair rows: [x(2i) | x(2i+1) | q(2i) | q(2i+1)]
    xq4 = np.zeros((cfg.npair, 4, P), BF)
    xn = np.zeros((cfg.npos, P), BF)
    xn[m_sl] = x[slotnode[m_sl]].astype(BF)
    xq4[:, 0, :] = xn[0::2]
    xq4[:, 1, :] = xn[1::2]
    xq = np.ascontiguousarray(xq4.reshape(cfg.npair, 4 * P))

    w1n = np.asarray(w1n, np.float32)
    w1r = np.asarray(w1r, np.float32)
    w2n = np.asarray(w2n, np.float32)
    w2r = np.asarray(w2r, np.float32)
    iota = np.tile(np.arange(2 * P, dtype=np.float32), (P, 1))
    shared = {
        "xq": xq,
        "w1nT": np.ascontiguousarray(w1n.T.astype(BF)),
        "w1rT": np.ascontiguousarray(w1r.T.astype(BF)),
        "b1c": np.ascontiguousarray(np.asarray(b1, np.float32).reshape(2, P).T),
        "w2nT": np.ascontiguousarray(
            np.concatenate([w2n.T[:P, :], w2n.T[P:, :]], axis=1).astype(BF)),
        "w2rT": np.ascontiguousarray(
            np.concatenate([w2r.T[:P, :], w2r.T[P:, :]], axis=1).astype(BF)),
        "b2r": np.ascontiguousarray(np.asarray(b2, np.float32).reshape(1, P)
                                    .astype(BF)),
        "iota2": np.ascontiguousarray(iota.astype(BF)),
        "ident": np.ascontiguousarray(np.eye(P, dtype=np.float32).astype(BF)),
    }

    in_maps = []
    for c in range(cfg.n_cores):
        m = core == c
        etl, edi = tl[m], dval[m]
        epair, eearly = pairidx[m], early[m]

        so = np.argsort(etl * 2 + (~eearly).astype(np.int64), kind="stable")
        s_tl, s_early = etl[so], eearly[so]
        seg = s_tl * 2 + (~s_early).astype(np.int64)
        rank = _rank_in_groups(seg, 2 * nt)
        slot1 = np.where(s_early, ebase[s_tl] + rank, lbase[s_tl] + rank)
        eidx1 = np.zeros(tot1, np.int64)
        dloc1 = np.full(tot1, -1.0, np.float32)
        BeR_, BlR_ = rup(Be), rup(Bl)
        for grp in cfg.groups:
            tl_ = grp[-1]
            eidx1[ebase[tl_] + Be[tl_]:ebase[tl_] + BeR_[tl_]] = -1
            eidx1[lbase[tl_] + Bl[tl_]:lbase[tl_] + BlR_[tl_]] = -1
        eidx1[slot1] = epair[so]
        dloc1[slot1] = edi[so]

        lm = ~s_early
        slot2 = base2[s_tl[lm]] + rank[lm]
        eidx2 = np.zeros(tot2, np.int64)
        dloc2 = np.full(tot2, -1.0, np.float32)
        for grp in g2grps:
            tl_ = grp[-1]
            eidx2[base2[tl_] + Bl[tl_]:base2[tl_] + B2[tl_]] = -1
        eidx2[slot2] = epair[so][lm]
        dloc2[slot2] = edi[so][lm]

        sl = slotnode[c * npad:(c + 1) * npad]
        msl = sl >= 0
        xt = np.zeros((P, npad), BF)
        xt[:, msl] = x[sl[msl]].T.astype(BF)
        dinv = np.zeros((P, nt), np.float32)
        dinv.T.flat[msl] = deginv[sl[msl]]

        im = dict(shared)
        im["eidx1"] = np.ascontiguousarray(
            np.tile(eidx1.astype(np.int16).reshape(-1, 16).T, (8, 1)))
        im["dloc1"] = np.ascontiguousarray(dloc1.reshape(-1, P).T.astype(BF))
        im["eidx2"] = np.ascontiguousarray(
            np.tile(eidx2.astype(np.int16).reshape(-1, 16).T, (8, 1)))
        im["dloc2"] = np.ascontiguousarray(dloc2.reshape(-1, P).T.astype(BF))
        im["xtile"] = xt
        im["dinv"] = dinv
        in_maps.append(im)

    budgets = (tuple(int(v) for v in Be), tuple(int(v) for v in Bl))
    return budgets, in_maps


# --------------------------------------------------------------------------
# Bass program
# --------------------------------------------------------------------------

def build_program(cfg, budgets):
    Be = np.array(budgets[0], np.int64)
    Bl = np.array(budgets[1], np.int64)
    BeR, BlR = rup(Be), rup(Bl)
    ebase, lbase, spans, tot1 = slot_layout(cfg, Be, Bl)
    B2 = rup(Bl)
    nt, npad, W = cfg.nt, cfg.npad, cfg.W
    _, _g2, base2 = pass2_order_groups(nt, B2)
    tot2 = int(base2[nt])

    nc = bacc.Bacc("TRN2", target_bir_lowering=False, debug=False,
                   enable_asserts=False, num_devices=cfg.n_cores)

    xq_d = nc.dram_tensor("xq", [cfg.npair, 4 * P], BF16, kind="ExternalInput")
    e1_d = nc.dram_tensor("eidx1", [P, tot1 // 16], I16, kind="ExternalInput")
    d1_d = nc.dram_tensor("dloc1", [P, tot1 // P], BF16, kind="ExternalInput")
    e2_d = nc.dram_tensor("eidx2", [P, tot2 // 16], I16, kind="ExternalInput")
    d2_d = nc.dram_tensor("dloc2", [P, tot2 // P], BF16, kind="ExternalInput")
    xt_d = nc.dram_tensor("xtile", [P, npad], BF16, kind="ExternalInput")
    dinv_d = nc.dram_tensor("dinv", [P, nt], F32, kind="ExternalInput")
    w1n_d = nc.dram_tensor("w1nT", [P, cfg.hid], BF16, kind="ExternalInput")
    w1r_d = nc.dram_tensor("w1rT", [P, cfg.hid], BF16, kind="ExternalInput")
    b1_d = nc.dram_tensor("b1c", [P, 2], F32, kind="ExternalInput")
    w2n_d = nc.dram_tensor("w2nT", [P, 2 * P], BF16, kind="ExternalInput")
    w2r_d = nc.dram_tensor("w2rT", [P, 2 * P], BF16, kind="ExternalInput")
    b2_d = nc.dram_tensor("b2r", [1, P], BF16, kind="ExternalInput")
    iota_d = nc.dram_tensor("iota2", [P, 2 * P], BF16, kind="ExternalInput")
    id_d = nc.dram_tensor("ident", [P, P], BF16, kind="ExternalInput")
    out_d = nc.dram_tensor("out", [npad, P], F32, kind="ExternalOutput")

    AF = mybir.ActivationFunctionType
    OP = mybir.AluOpType

    # max gather-buffer size (bf16 elems per partition): early rows (4P
    # elems each) plus late x-halves (2P) share one pooled buffer
    max_g1 = max(es * 4 + ls * 2 for (_, es, _, ls) in spans)
    _, g2sizes, base2 = pass2_order_groups(nt, B2)
    max_g2 = max(sum(int(B2[u]) for u in grp) for grp in g2sizes) * 2

    with tile.TileContext(nc) as tc:
        with (tc.tile_pool(name="const", bufs=1) as cp,
              tc.tile_pool(name="dram", bufs=1, space="DRAM") as dp,
              tc.tile_pool(name="oh", bufs=8) as ohp,
              tc.tile_pool(name="stage", bufs=6) as sp):
            eidx1 = cp.tile([P, tot1 // 16], I16)
            nc.sync.dma_start(out=eidx1[:], in_=e1_d.ap())
            dloc1 = cp.tile([P, tot1 // P], BF16)
            nc.sync.dma_start(out=dloc1[:], in_=d1_d.ap())
            iota2 = cp.tile([P, 2 * P], BF16)
            nc.sync.dma_start(out=iota2[:], in_=iota_d.ap())
            ident = cp.tile([P, P], BF16)
            nc.sync.dma_start(out=ident[:], in_=id_d.ap())
            xt = cp.tile([P, npad], BF16)
            nc.sync.dma_start(out=xt[:], in_=xt_d.ap())
            dinv = cp.tile([P, nt], F32)
            nc.sync.dma_start(out=dinv[:], in_=dinv_d.ap())
            w1n = cp.tile([P, cfg.hid], BF16)
            nc.sync.dma_start(out=w1n[:], in_=w1n_d.ap())
            w1r = cp.tile([P, cfg.hid], BF16)
            nc.sync.dma_start(out=w1r[:], in_=w1r_d.ap())
            b1c = cp.tile([P, 2], F32)
            nc.sync.dma_start(out=b1c[:], in_=b1_d.ap())
            w2n = cp.tile([P, 2 * P], BF16)
            nc.sync.dma_start(out=w2n[:], in_=w2n_d.ap())
            w2r = cp.tile([P, 2 * P], BF16)
            nc.sync.dma_start(out=w2r[:], in_=w2r_d.ap())
            b2r = cp.tile([1, P], BF16)
            nc.sync.dma_start(out=b2r[:], in_=b2_d.ap())
            eidx2 = cp.tile([P, tot2 // 16], I16)
            nc.sync.dma_start(out=eidx2[:], in_=e2_d.ap())
            dloc2 = cp.tile([P, tot2 // P], BF16)
            nc.sync.dma_start(out=dloc2[:], in_=d2_d.ap())
            ones1 = cp.tile([1, P], BF16)
            nc.vector.memset(ones1[:], 1.0)
            hT = cp.tile([P, 2 * npad], BF16)
            stash = cp.tile([P, nt * P], BF16)

            warm_in = dp.tile([P, P], BF16, name="warm_in")
            warm_out = dp.tile([cfg.n_cores * P, P], BF16, name="warm_out",
                               addr_space="Shared")
            q_locs, q_alls = [], []
            for w in range(W):
                rows = len(cfg.wave_tiles[w]) * P
                q_locs.append(dp.tile([rows, P], BF16, name=f"qloc{w}"))
                q_alls.append(dp.tile([cfg.n_cores * rows, P], BF16,
                                      name=f"qall{w}", addr_space="Shared"))

            def onehot(dloc_t, gc):
                oh = ohp.tile([P, 2 * P], BF16, tag="oh")
                nc.vector.tensor_tensor(
                    out=oh[:], in0=iota2[:],
                    in1=dloc_t[:, gc:gc + 1].to_broadcast([P, 2 * P]),
                    op=OP.is_equal)
                return oh

            def emit_repack(w):
                """q_all[w] rows -> q-halves of xq pair rows (both parities).
                q(pos) lives at pair pos>>1, elem offset (2+(pos&1))*128."""
                rows = len(cfg.wave_tiles[w]) * P
                t0 = int(cfg.wave_tiles[w][0])
                pr0 = t0 * P // 2
                npr = rows // 2
                qa = q_alls[w][:, :].rearrange("(c j t) e -> c j (t e)",
                                               c=cfg.n_cores, t=2)
                xqv = xq_d.ap().rearrange("(c q) e -> c q e", c=cfg.n_cores)
                for parity in range(2):
                    nc.sync.dma_start(
                        out=xqv[:, pr0:pr0 + npr,
                                (2 + parity) * P:(3 + parity) * P],
                        in_=qa[:, :, parity * P:(parity + 1) * P])

            def emit_ag(w):
                nc.gpsimd.collective_compute(
                    "AllGather", mybir.AluOpType.bypass,
                    replica_groups=[list(range(cfg.n_cores))],
                    ins=[q_locs[w].opt()],
                    outs=[q_alls[w].opt()])

            xq_full = xq_d.ap()                              # [npair, 512]
            xq_x = xq_d.ap()[:, 0:2 * P]                     # x halves
            xq_q = xq_d.ap()[:, 2 * P:4 * P]                 # q halves

            def tile_pass1(t, ge, ge0, gl, gl0, pp, qrow, is_last):
                ce = int(BeR[t]) // P
                cl = int(BlR[t]) // P
                ne = int(Be[t])
                Ke = int(Be[t]) - P * (ce - 1) if is_last and ce else P
                Kl = int(Bl[t]) - P * (cl - 1) if is_last and cl else P
                ps1 = pp.tile([P, P], F32, tag="ps1", name=f"ps1_{t}")
                ps2 = None
                if ne > 0:
                    ps2 = pp.tile([P, P], F32, tag="ps2", name=f"ps2_{t}")
                # early chunks: full pair rows [x0|x1|q0|q1]
                for j in range(ce):
                    gc = (int(ebase[t])) // P + j
                    lc = (int(ebase[t]) - ge0) // P + j
                    K = Ke if j == ce - 1 else P
                    oh = onehot(dloc1, gc)
                    nc.tensor.matmul(ps1[:], lhsT=oh[0:K, 0:P],
                                     rhs=ge[0:K, lc * 4 * P:lc * 4 * P + P],
                                     start=(j == 0), stop=False)
                    nc.tensor.matmul(ps1[:], lhsT=oh[0:K, P:2 * P],
                                     rhs=ge[0:K, lc * 4 * P + P:lc * 4 * P + 2 * P],
                                     start=False, stop=False)
                    nc.tensor.matmul(ps2[:], lhsT=oh[0:K, 0:P],
                                     rhs=ge[0:K, lc * 4 * P + 2 * P:
                                           lc * 4 * P + 3 * P],
                                     start=(j == 0), stop=False)
                    nc.tensor.matmul(ps2[:], lhsT=oh[0:K, P:2 * P],
                                     rhs=ge[0:K, lc * 4 * P + 3 * P:
                                           lc * 4 * P + 4 * P],
                                     start=False, stop=(j == ce - 1))
                # late chunks: x halves only [x0|x1]
                for j in range(cl):
                    gc = (int(lbase[t])) // P + j
                    lc = (int(lbase[t]) - gl0) // P + j
                    K = Kl if j == cl - 1 else P
                    oh = onehot(dloc1, gc)
                    nc.tensor.matmul(ps1[:], lhsT=oh[0:K, 0:P],
                                     rhs=gl[0:K, lc * 2 * P:lc * 2 * P + P],
                                     start=(ce == 0 and j == 0), stop=False)
                    nc.tensor.matmul(ps1[:], lhsT=oh[0:K, P:2 * P],
                                     rhs=gl[0:K, lc * 2 * P + P:lc * 2 * P + 2 * P],
                                     start=False, stop=(j == cl - 1))
                agg_nm = sp.tile([P, P], BF16, tag="aggnm")
                nc.scalar.activation(agg_nm[:], ps1[:], AF.Copy,
                                     scale=dinv[:, t:t + 1])
                ps_t = pp.tile([P, P], BF16, tag="pst", name=f"pst{t}", bufs=1)
                nc.tensor.transpose(ps_t[:], agg_nm[:], ident[:])
                aggrT = sp.tile([P, P], BF16, tag="aggrT")
                nc.scalar.activation(aggrT[:], ps_t[:], AF.Copy)
                for h in range(2):
                    ps_h = pp.tile([P, P], F32, tag="psh", name=f"psh{t}_{h}")
                    nc.tensor.matmul(ps_h[:], lhsT=w1n[:, h * P:(h + 1) * P],
                                     rhs=aggrT[:], start=True, stop=False)
                    nc.tensor.matmul(ps_h[:], lhsT=w1r[:, h * P:(h + 1) * P],
                                     rhs=xt[:, t * P:(t + 1) * P],
                                     start=False, stop=True)
                    nc.scalar.activation(
                        hT[:, h * npad + t * P:h * npad + (t + 1) * P],
                        ps_h[:], AF.Relu, bias=b1c[:, h:h + 1])
                ps_q = pp.tile([P, P], F32, tag="psq", name=f"psq{t}", bufs=1)
                nc.tensor.matmul(ps_q[:], lhsT=hT[:, t * P:(t + 1) * P],
                                 rhs=w2n[:, 0:P], start=True, stop=False)
                nc.tensor.matmul(ps_q[:],
                                 lhsT=hT[:, npad + t * P:npad + (t + 1) * P],
                                 rhs=w2n[:, P:2 * P], start=False, stop=True)
                qsb = sp.tile([P, P], BF16, tag="qsb")
                nc.scalar.activation(qsb[:], ps_q[:], AF.Copy)
                w = int(cfg.wave_of[t])
                nc.scalar.dma_start(out=q_locs[w][qrow:qrow + P, :], in_=qsb[:])
                if ne > 0:
                    nc.scalar.activation(stash[:, t * P:(t + 1) * P],
                                         ps2[:], AF.Copy)

            # ------------------- pass 1: waves -------------------
            nc.gpsimd.collective_compute(
                "AllGather", mybir.AluOpType.bypass,
                replica_groups=[list(range(cfg.n_cores))],
                ins=[warm_in.opt()], outs=[warm_out.opt()])
            with (tc.tile_pool(name="gath", bufs=3) as gp,
                  tc.tile_pool(name="ps1p", bufs=2, space="PSUM") as pp1):
                gidx = 0
                for w in range(W):
                    first = True
                    for grp in [g for g in cfg.groups
                                if cfg.wave_of[g[0]] == w]:
                        e0, esz, l0, lsz = spans[gidx]
                        gidx += 1
                        gb = gp.tile([P, max_g1], BF16, tag="g")
                        ge = gb[:, :esz * 4] if esz else None
                        gl = gb[:, esz * 4:esz * 4 + lsz * 2]
                        if esz:
                            nc.gpsimd.dma_gather(
                                ge.rearrange("p (c e) -> p c e", e=4 * P),
                                xq_full, eidx1[:, e0 // 16:(e0 + esz) // 16],
                                esz, esz, 4 * P, single_packet=False)
                        nc.gpsimd.dma_gather(
                            gl.rearrange("p (c e) -> p c e", e=2 * P),
                            xq_x, eidx1[:, l0 // 16:(l0 + lsz) // 16],
                            lsz, lsz, 2 * P,
                            elem_step=4 * P, single_packet=False)
                        for t in grp:
                            wt0 = int(cfg.wave_tiles[w][0])
                            tile_pass1(int(t), ge, e0, gl, l0, pp1,
                                       (int(t) - wt0) * P,
                                       int(t) == int(grp[-1]))
                        if first and w >= 1:
                            emit_ag(w - 1)
                            if w >= 2:
                                emit_repack(w - 2)
                        first = False
                emit_ag(W - 1)
                emit_repack(W - 2)
                emit_repack(W - 1)

            # ------------------- pass 2 -------------------
            with (tc.tile_pool(name="gath2", bufs=4) as gp2,
                  tc.tile_pool(name="ps2p", bufs=2, space="PSUM") as pp2):
                for grp in g2sizes:
                    s0 = int(base2[grp[0]])
                    nidx = int(sum(B2[u] for u in grp))
                    g2 = gp2.tile([P, max_g2], BF16, tag="g2")
                    nc.gpsimd.dma_gather(
                        g2[:, :nidx * 2].rearrange("p (c e) -> p c e", e=2 * P),
                        xq_q, eidx2[:, s0 // 16:(s0 + nidx) // 16],
                        nidx, nidx, 2 * P,
                        elem_step=4 * P, single_packet=False)
                    for t in grp:
                        ct = int(B2[t]) // P
                        K2l = int(Bl[t]) - P * (ct - 1) \
                            if t == grp[-1] else P
                        ps_b = pp2.tile([P, P], F32, tag="psb", name=f"psb{t}")
                        first = True
                        if Be[t] > 0:
                            nc.tensor.matmul(ps_b[:], lhsT=ident[:],
                                             rhs=stash[:, t * P:(t + 1) * P],
                                             start=True, stop=False)
                            first = False
                        for j in range(ct):
                            gc = int(base2[t]) // P + j
                            lc = (int(base2[t]) - s0) // P + j
                            K = K2l if j == ct - 1 else P
                            oh = onehot(dloc2, gc)
                            nc.tensor.matmul(
                                ps_b[:], lhsT=oh[0:K, 0:P],
                                rhs=g2[0:K, lc * 2 * P:lc * 2 * P + P],
                                start=first, stop=False)
                            first = False
                            nc.tensor.matmul(
                                ps_b[:], lhsT=oh[0:K, P:2 * P],
                                rhs=g2[0:K, lc * 2 * P + P:lc * 2 * P + 2 * P],
                                start=False, stop=(j == ct - 1))
                        agg_sb = sp.tile([P, P], F32, tag="aggsb")
                        nc.scalar.activation(agg_sb[:], ps_b[:], AF.Copy,
                                             scale=dinv[:, t:t + 1])
                        ps_r = pp2.tile([P, P], F32, tag="psr", name=f"psr{t}")
                        nc.tensor.matmul(ps_r[:], lhsT=hT[:, t * P:(t + 1) * P],
                                         rhs=w2r[:, 0:P], start=True, stop=False)
                        nc.tensor.matmul(
                            ps_r[:], lhsT=hT[:, npad + t * P:npad + (t + 1) * P],
                            rhs=w2r[:, P:2 * P], start=False, stop=False)
                        nc.tensor.matmul(ps_r[:], lhsT=ones1[:], rhs=b2r[:],
                                         start=False, stop=True)
                        osb = sp.tile([P, P], F32, tag="osb")
                        nc.vector.tensor_tensor(out=osb[:], in0=agg_sb[:],
                                                in1=ps_r[:], op=OP.add)
                        nc.sync.dma_start(out=out_d.ap()[t * P:(t + 1) * P, :],
                                          in_=osb[:])

    nc.compile()
    return nc


# --------------------------------------------------------------------------
# entry point
# --------------------------------------------------------------------------

_CACHE = {}


def prepare(inputs, cfg=None):
    x = np.asarray(inputs["x"], np.float32)
    if cfg is None:
        cfg = Cfg(n=x.shape[0])
    budgets, in_maps = pack_inputs(
        cfg, x, inputs["edge_index"],
        inputs["W1_nbr"], inputs["W1_root"], inputs["b1"],
        inputs["W2_nbr"], inputs["W2_root"], inputs["b2"])
    key = (cfg.key(), budgets)
    nc = _CACHE.get(key)
    if nc is None:
        nc = build_program(cfg, budgets)
        _CACHE[key] = nc
    return nc, in_maps, cfg


def kernel(**inputs) -> np.ndarray:
    nc, in_maps, cfg = prepare(inputs)
    res = bass_utils.run_bass_kernel_spmd(
        nc, in_maps, core_ids=list(range(cfg.n_cores)))
    out = np.concatenate([res.results[c]["out"] for c in range(cfg.n_cores)],
                         axis=0)
    return np.ascontiguousarray(out[cfg.newpos], dtype=np.float32)
